# revision 22
# baseline (speedup 1.0000x reference)
"""BiMamba (bidirectional Mamba v1) TRN2 Bass kernel, 8-core SPMD. v2.

Shapes (hardcoded): B=1, L=2048, Dm=1024, Di=2048, N=16, K=4, R=64.
Sharding: tensor-parallel over d_inner across 8 cores (Di_loc=256).
Collectives: per-direction AllReduce of x_dbl (f32), column-chunked
ReduceScatter of out^T (bf16).

Engine plan (measured rates): DVE owns the 64 time-scans (4.42us each,
irreducible) plus the two bf16 2x multiplies per scan tile (dBx, hc).
ACT owns exp/softplus/silu and all PSUM->SBUF bf16 copies. PE owns every
matmul including the depthwise conv (diag lhsT), dt/dtx packed broadcasts,
the D-skip terms and the state contraction. Pool only triggers casts and
collectives (DVE+Pool share SBUF ports; concurrent Pool compute is a wash).

Scan layout: packed partitions p = 16*s + n (s = channel-in-octet 0..8,
n = state 0..16); 16 scan tiles per 128-channel group; time on the free
axis; the reverse direction is the same scan with negative strides.
"""
import sys

sys.path.insert(0, '/opt/trn_rl_repo')

import ml_dtypes
import numpy as np

import concourse.bacc as bacc
import concourse.mybir as mybir
from concourse.bass_utils import run_bass_kernel_spmd
from concourse.tile import TileContext

AF = mybir.ActivationFunctionType
ALU = mybir.AluOpType
F32 = mybir.dt.float32
BF16 = mybir.dt.bfloat16
BF16NP = ml_dtypes.bfloat16

L = 2048
DM = 1024
DI = 2048
N = 16
KC = 4
R = 64
NCORES = 8
DLOC = DI // NCORES      # 256
NG = DLOC // 128         # 2
NT = 16                  # scan tiles per 128-channel group
PADL = KC - 1            # 3
XPW = L + 2 * PADL
HC = 1024                # PSUM chunk cols (2 banks)
NHC = L // HC            # 2


_CACHE = {}


def _emit(nc, t):
    with TileContext(nc) as tc:
        with tc.tile_pool(name='sb', bufs=1) as P, \
             tc.tile_pool(name='ps', bufs=1, space='PSUM') as PS:

            # ---------------- loads (queue-spread) ----------------
            hks = []
            for k in range(8):
                hk = P.tile([128, L], BF16, tag='tr', bufs=13, name=f'hk{k}')
                nc.sync.dma_start(out=hk[:], in_=t['h_T'][k * 128:(k + 1) * 128, :])
                hks.append(hk)
            win_k = []
            for k in range(8):
                wk = P.tile([128, 512], BF16, tag='win', bufs=8, name=f'win{k}')
                nc.scalar.dma_start(out=wk[:],
                                    in_=t['w_in_T'][k * 128:(k + 1) * 128, :])
                win_k.append(wk)

            def vload(name, src, rows, cols):
                tl = P.tile([rows, cols], BF16, tag=name, bufs=1, name=name)
                nc.gpsimd.dma_start(out=tl[:], in_=src[:])
                return tl

            sel01m = vload('sel01', t['sel01'], 128, NT * 128)
            gselm = vload('gsel', t['gsel'], 128, NT * 128)
            ddiagm = vload('ddiag', t['ddiag'], 128, 4 * 128)
            selBCm = vload('selBC', t['selBC'], 32, 256)
            wxm = vload('wx', t['wx_T'], 128, 384)   # cols g*192 + d*96
            wdt_t = vload('wdt', t['wdt_T'], R, 2 * DLOC)
            wom = vload('wo', t['w_out_T'], 128, 2 * DM)  # [g, mb-cols]

            def sel01j(j):
                return sel01m[:, j * 128:(j + 1) * 128]

            def gselj(j):
                return gselm[:, j * 128:(j + 1) * 128]

            apack = P.tile([128, 64], F32, tag='apack', bufs=1, name='apack')
            nc.sync.dma_start(out=apack[:], in_=t['apack'][:])
            bdtb = P.tile([128, 4], F32, tag='bdtb', bufs=1, name='bdtb')
            nc.sync.dma_start(out=bdtb[:], in_=t['bdt'][:])
            convb = P.tile([128, 4], F32, tag='convb', bufs=1, name='convb')
            nc.sync.dma_start(out=convb[:], in_=t['conv_b'][:])
            convw = P.tile([128, 16], F32, tag='convw', bufs=1, name='convw')
            nc.sync.dma_start(out=convw[:], in_=t['conv_w'][:])

            x_pad = []
            z_t = []
            for g in range(NG):
                xp = P.tile([128, XPW], BF16, tag='xpad', bufs=2, name=f'xp{g}')
                nc.gpsimd.memset(xp[:, :PADL], 0.0)
                nc.gpsimd.memset(xp[:, PADL + L:], 0.0)
                x_pad.append(xp)
                zt = P.tile([128, L], BF16, tag='z', bufs=2, name=f'z{g}')
                z_t.append(zt)

            def in_proj_block(ob):
                g = ob % 2
                is_x = ob < 2
                col0 = (g * 128) if is_x else (256 + g * 128)
                for c in range(NHC):
                    pi = PS.tile([128, HC], F32, tag='psm', bufs=2,
                                 name=f'pi{ob}_{c}')
                    for h2 in range(2):
                        sl = slice(c * HC + h2 * 512, c * HC + (h2 + 1) * 512)
                        osl = slice(h2 * 512, (h2 + 1) * 512)
                        for k in range(8):
                            nc.tensor.matmul(
                                out=pi[:, osl],
                                lhsT=win_k[k][:, col0:col0 + 128],
                                rhs=hks[k][:, sl],
                                start=(k == 0), stop=(k == 7))
                    if is_x:
                        nc.scalar.activation(
                            out=x_pad[g][:, PADL + c * HC:PADL + (c + 1) * HC],
                            in_=pi[:], func=AF.Copy)
                    else:
                        nc.scalar.activation(
                            out=z_t[g][:, c * HC:(c + 1) * HC],
                            in_=pi[:], func=AF.Copy)

            xa = {}

            def conv_block(d):
                base = 0 if d == 0 else PADL
                for g in range(NG):
                    col = (d * NG + g) * KC
                    acc = P.tile([128, L], BF16, tag='tr', bufs=13,
                                 name=f'cv0_{d}{g}')
                    nc.vector.tensor_scalar(
                        out=acc[:], in0=x_pad[g][:, base:base + L],
                        scalar1=convw[:, col:col + 1], scalar2=None,
                        op0=ALU.mult)
                    for k in range(1, KC):
                        acc2 = P.tile([128, L], BF16, tag='tr', bufs=13,
                                      name=f'cv{k}_{d}{g}')
                        nc.vector.scalar_tensor_tensor(
                            out=acc2[:], in0=x_pad[g][:, base + k:base + k + L],
                            scalar=convw[:, col + k:col + k + 1],
                            in1=acc[:], op0=ALU.mult, op1=ALU.add)
                        acc = acc2
                    xat = P.tile([128, L], BF16, tag='xa', bufs=4,
                                 name=f'xa{d}{g}')
                    nc.scalar.activation(
                        out=xat[:], in_=acc[:], func=AF.Silu,
                        bias=convb[:, 2 * d + g:2 * d + g + 1])
                    xa[(d, g)] = xat

            def wx_ar_block(d):
                wxs = P.tile([96, L], F32, tag='wxs', bufs=1, name=f'wxs{d}')
                for c in range(NHC):
                    pw = PS.tile([96, HC], F32, tag='psm', bufs=2,
                                 name=f'pw{d}{c}')
                    for h2 in range(2):
                        o0 = c * HC + h2 * 512
                        osl = slice(h2 * 512, (h2 + 1) * 512)
                        for g in range(NG):
                            nc.tensor.matmul(
                                out=pw[:, osl],
                                lhsT=wxm[:, g * 192 + 96 * d:
                                         g * 192 + 96 * (d + 1)],
                                rhs=xa[(d, g)][:, o0:o0 + 512],
                                start=(g == 0), stop=(g == NG - 1))
                    nc.scalar.activation(out=wxs[:, c * HC:(c + 1) * HC],
                                         in_=pw[:], func=AF.Copy)
                nc.sync.dma_start(out=t['ar_in'][96 * d:96 * (d + 1), :],
                                  in_=wxs[:])
                nc.gpsimd.collective_compute(
                    'AllReduce', ALU.add, replica_groups=[list(range(NCORES))],
                    ins=[t['ar_in'][96 * d:96 * (d + 1), :]],
                    outs=[t['ar_out'][96 * d:96 * (d + 1), :]])

            dt_t = {}
            dtx_t = {}
            bc = {}

            def prep_block(d):
                dtraw = P.tile([R, L], BF16, tag='dtraw', bufs=2,
                               name=f'dtraw{d}')
                nc.gpsimd.dma_start(
                    out=dtraw[:], in_=t['ar_out'][96 * d:96 * d + 64, :])
                arbc = P.tile([32, L], BF16, tag='arbc', bufs=2,
                              name=f'arbc{d}')
                nc.gpsimd.dma_start(
                    out=arbc[:], in_=t['ar_out'][96 * d + 64:96 * d + 96, :])
                for bi, nm in ((0, 'B'), (1, 'C')):
                    tile = P.tile([128, L], BF16, tag='bc', bufs=4,
                                  name=f'{nm}bc{d}')
                    for c in range(NHC):
                        pb = PS.tile([128, HC], F32, tag='psm', bufs=2,
                                     name=f'pb{d}{bi}{c}')
                        for h2 in range(2):
                            o0 = c * HC + h2 * 512
                            osl = slice(h2 * 512, (h2 + 1) * 512)
                            nc.tensor.matmul(
                                out=pb[:, osl],
                                lhsT=selBCm[:, bi * 128:(bi + 1) * 128],
                                rhs=arbc[:, o0:o0 + 512],
                                start=True, stop=True)
                        nc.scalar.activation(out=tile[:, c * HC:(c + 1) * HC],
                                             in_=pb[:], func=AF.Copy)
                    bc[(d, nm)] = tile
                for g in range(NG):
                    ett = P.tile([128, L], BF16, tag='tr', bufs=13,
                                 name=f'et{d}{g}')
                    for c in range(NHC):
                        pe = PS.tile([128, HC], F32, tag='psm', bufs=2,
                                     name=f'pe{d}{g}{c}')
                        for h2 in range(2):
                            o0 = c * HC + h2 * 512
                            osl = slice(h2 * 512, (h2 + 1) * 512)
                            nc.tensor.matmul(
                                out=pe[:, osl],
                                lhsT=wdt_t[:, d * DLOC + g * 128:
                                           d * DLOC + (g + 1) * 128],
                                rhs=dtraw[:, o0:o0 + 512],
                                start=True, stop=True)
                        nc.scalar.activation(
                            out=ett[:, c * HC:(c + 1) * HC], in_=pe[:],
                            func=AF.Exp,
                            bias=bdtb[:, 2 * d + g:2 * d + g + 1])
                    dtt = P.tile([128, L], BF16, tag='dt', bufs=4,
                                 name=f'dt{d}{g}')
                    nc.scalar.activation(out=dtt[:], in_=ett[:], func=AF.Ln,
                                         bias=1.0)
                    dt_t[(d, g)] = dtt
                    dtxt = P.tile([128, L], BF16, tag='dtx', bufs=4,
                                  name=f'dtx{d}{g}')
                    nc.vector.tensor_tensor(out=dtxt[:], in0=dtt[:],
                                            in1=xa[(d, g)][:], op=ALU.mult)
                    dtx_t[(d, g)] = dtxt

            # gsel contraction matmuls are deferred DEFER tiles so the PE
            # queue never blocks on the current tile's DVE chain.
            DEFER = 3
            py_t = {}
            pend = {}
            firsty = {}

            def flush_y(g, keep):
                if py_t.get(g) is None:
                    py_t[g] = PS.tile([128, L], F32, tag='py', bufs=1,
                                      name=f'py{g}')
                py = py_t[g]
                pending = pend[g]
                while len(pending) > keep:
                    jj, hct = pending.pop(0)
                    for c4 in range(4):
                        nc.tensor.matmul(
                            out=py[:, c4 * 512:(c4 + 1) * 512],
                            lhsT=gselj(jj),
                            rhs=hct[:, c4 * 512:(c4 + 1) * 512],
                            start=firsty[g], stop=False)
                    firsty[g] = False

            def scan_block(g, d):
                if d == 0:
                    py_t[g] = None
                    pend[g] = []
                    firsty[g] = True
                for j in range(NT):
                    colA = (d * NG + g) * NT + j
                    dA = P.tile([128, L], BF16, tag='tr', bufs=13,
                                name=f'dA{g}{d}{j}')
                    pxs = P.tile([128, L], BF16, tag='tr', bufs=13,
                                 name=f'pxs{g}{d}{j}')
                    for c in range(NHC):
                        pq = PS.tile([128, HC], F32, tag='psm', bufs=2,
                                     name=f'pq{g}{d}{j}{c}')
                        for h2 in range(2):
                            o0 = c * HC + h2 * 512
                            osl = slice(h2 * 512, (h2 + 1) * 512)
                            nc.tensor.matmul(
                                out=pq[:, osl], lhsT=sel01j(j),
                                rhs=dt_t[(d, g)][:, o0:o0 + 512],
                                start=True, stop=True)
                        nc.scalar.activation(
                            out=dA[:, c * HC:(c + 1) * HC], in_=pq[:],
                            func=AF.Exp,
                            scale=apack[:, colA:colA + 1])
                    dBx = P.tile([128, L], BF16, tag='tr', bufs=13,
                                 name=f'dBx{g}{d}{j}')
                    for c in range(NHC):
                        px = PS.tile([128, HC], F32, tag='psm', bufs=2,
                                     name=f'px{g}{d}{j}{c}')
                        for h2 in range(2):
                            o0 = c * HC + h2 * 512
                            osl = slice(h2 * 512, (h2 + 1) * 512)
                            nc.tensor.matmul(
                                out=px[:, osl], lhsT=sel01j(j),
                                rhs=dtx_t[(d, g)][:, o0:o0 + 512],
                                start=True, stop=True)
                        nc.scalar.activation(
                            out=pxs[:, c * HC:(c + 1) * HC], in_=px[:],
                            func=AF.Copy)
                    nc.vector.tensor_tensor(
                        out=dBx[:], in0=pxs[:], in1=bc[(d, 'B')][:],
                        op=ALU.mult)
                    h_t = P.tile([128, L], BF16, tag='tr', bufs=13,
                                 name=f'h{g}{d}{j}')
                    if d == 0:
                        nc.vector.tensor_tensor_scan(
                            out=h_t[:], data0=dA[:], data1=dBx[:],
                            initial=0.0, op0=ALU.mult, op1=ALU.add)
                    else:
                        nc.vector.tensor_tensor_scan(
                            out=h_t[:, ::-1], data0=dA[:, ::-1],
                            data1=dBx[:, ::-1],
                            initial=0.0, op0=ALU.mult, op1=ALU.add)
                    hc_t = P.tile([128, L], BF16, tag='tr', bufs=13,
                                  name=f'hc{g}{d}{j}')
                    nc.vector.tensor_tensor(
                        out=hc_t[:], in0=h_t[:], in1=bc[(d, 'C')][:],
                        op=ALU.mult)
                    pend[g].append((j, hc_t))
                    flush_y(g, DEFER)

            def gates_block(g):
                flush_y(g, 0)
                py = py_t[g]
                for d in range(2):
                    for c4 in range(4):
                        nc.tensor.matmul(
                            out=py[:, c4 * 512:(c4 + 1) * 512],
                            lhsT=ddiagm[:, (d * NG + g) * 128:
                                        (d * NG + g + 1) * 128],
                            rhs=xa[(d, g)][:, c4 * 512:(c4 + 1) * 512],
                            start=False, stop=(d == 1))
                sg = P.tile([128, L], BF16, tag='tr', bufs=13, name=f'sg{g}')
                nc.scalar.activation(out=sg[:], in_=z_t[g][:], func=AF.Silu)
                yg = P.tile([128, L], BF16, tag='yg', bufs=2, name=f'yg{g}')
                for c4 in range(4):
                    csl = slice(c4 * 512, (c4 + 1) * 512)
                    nc.vector.scalar_tensor_tensor(
                        out=yg[:, csl], in0=py[:, csl], scalar=1.0,
                        in1=sg[:, csl], op0=ALU.mult, op1=ALU.mult)
                t[f'_yg{g}'] = yg

            # ---------------- phase schedule ----------------
            in_proj_block(0)
            in_proj_block(1)
            conv_block(0)
            wx_ar_block(0)
            in_proj_block(2)   # z projections fill the AR0 latency
            in_proj_block(3)
            conv_block(1)
            prep_block(0)
            wx_ar_block(1)
            scan_block(0, 0)
            prep_block(1)      # d=1 prep overlaps the g0/d0 scans
            scan_block(0, 1)
            gates_block(0)
            scan_block(1, 0)
            scan_block(1, 1)
            gates_block(1)

            # ---------------- out_proj + chunked ReduceScatter ----------------
            OC = 1024
            for c in range(L // OC):
                for mb in range(8):
                    po = PS.tile([128, OC], F32, tag='psm', bufs=2,
                                 name=f'po{c}{mb}')
                    for h2 in range(2):
                        osl = slice(h2 * 512, (h2 + 1) * 512)
                        o0 = c * OC + h2 * 512
                        for g in range(NG):
                            nc.tensor.matmul(
                                out=po[:, osl],
                                lhsT=wom[:, g * DM + mb * 128:
                                         g * DM + (mb + 1) * 128],
                                rhs=t[f'_yg{g}'][:, o0:o0 + 512],
                                start=(g == 0), stop=(g == NG - 1))
                    ost = P.tile([128, OC], BF16, tag='ost', bufs=4,
                                 name=f'os{c}{mb}')
                    if mb % 2 == 0:
                        nc.vector.tensor_scalar(out=ost[:], in0=po[:],
                                                scalar1=1.0, scalar2=None,
                                                op0=ALU.mult)
                    else:
                        nc.scalar.activation(out=ost[:], in_=po[:],
                                             func=AF.Copy)
                    nc.sync.dma_start(
                        out=t[f'rs_in{c}'][mb * 128:(mb + 1) * 128, :],
                        in_=ost[:])
                nc.gpsimd.collective_compute(
                    'ReduceScatter', ALU.add,
                    replica_groups=[list(range(NCORES))],
                    ins=[t[f'rs_in{c}'][:]], outs=[t[f'rs_out{c}'][:]])
                nc.sync.dma_start(out=t['out_mT'][:, c * OC:(c + 1) * OC],
                                  in_=t[f'rs_out{c}'][:])


def _build():
    nc = bacc.Bacc(None, target_bir_lowering=False)

    def binp(name, shape):
        return nc.declare_dram_parameter(name, shape, BF16, isOutput=False)

    def finp(name, shape):
        return nc.declare_dram_parameter(name, shape, F32, isOutput=False)

    t = {
        'h_T': binp('h_T', [DM, L]),
        'w_in_T': binp('w_in_T', [DM, 512]),
        'wx_T': binp('wx_T', [128, 2 * 192]),
        'wdt_T': binp('wdt_T', [R, 2 * DLOC]),
        'w_out_T': binp('w_out_T', [128, 2 * DM]),
        'sel01': binp('sel01', [128, NT * 128]),
        'gsel': binp('gsel', [128, NT * 128]),
        'conv_w': finp('conv_w', [128, 16]),
        'ddiag': binp('ddiag', [128, 4 * 128]),
        'selBC': binp('selBC', [32, 256]),
        'apack': finp('apack', [128, 64]),
        'bdt': finp('bdt', [128, 4]),
        'conv_b': finp('conv_b', [128, 4]),
        'out_mT': nc.declare_dram_parameter('out_mT', [128, L], BF16,
                                            isOutput=True),
        'ar_in': nc.dram_tensor('ar_in', [192, L], F32),
        'ar_out': nc.dram_tensor('ar_out', [192, L], F32,
                                 addr_space='Shared'),
    }
    for c in range(2):
        t[f'rs_in{c}'] = nc.dram_tensor(f'rs_in{c}', [DM, 1024], BF16)
        t[f'rs_out{c}'] = nc.dram_tensor(f'rs_out{c}', [128, 1024], BF16)
    _emit(nc, t)
    nc.compile()
    return nc


def _sel_matrices():
    """sel01[j] (channel->packed broadcast), gsel[j] (state contraction)."""
    sel01 = np.zeros((NT, 128, 128), np.float32)
    gsel = np.zeros((NT, 128, 128), np.float32)
    for j in range(NT):
        for s in range(8):
            ch = 8 * j + s
            for n in range(N):
                sel01[j, ch, 16 * s + n] = 1.0
                gsel[j, 16 * s + n, ch] = 1.0
    selBC = np.zeros((2, 32, 128), np.float32)
    for s in range(8):
        for n in range(N):
            selBC[0, n, 16 * s + n] = 1.0       # B rows 0..16 of arbc
            selBC[1, 16 + n, 16 * s + n] = 1.0  # C rows 16..32 of arbc
    return sel01, gsel, selBC


def prepare_in_maps(inputs):
    f32 = np.float32
    h = np.asarray(inputs['hidden_states'], f32)[0]        # [L, DM]
    h_T = np.ascontiguousarray(h.T).astype(BF16NP)
    W_in = np.asarray(inputs['W_in'], f32)
    W_out = np.asarray(inputs['W_out'], f32)
    sel01, gsel, selBC = _sel_matrices()
    sel01 = np.concatenate(list(sel01), axis=1).astype(BF16NP)   # [128, NT*128]
    gsel = np.concatenate(list(gsel), axis=1).astype(BF16NP)
    selBC = np.concatenate(list(selBC), axis=1).astype(BF16NP)   # [32, 256]

    A = {0: -np.exp(np.asarray(inputs['A_log_fwd'], f32)),
         1: -np.exp(np.asarray(inputs['A_log_rev'], f32))}
    cw = {0: np.asarray(inputs['conv_w_fwd'], f32),
          1: np.asarray(inputs['conv_w_rev'], f32)[:, ::-1]}  # rev flipped

    maps = []
    for core in range(NCORES):
        sl = slice(core * DLOC, (core + 1) * DLOC)
        # apack[p=16s+n, col=(d*NG+g)*NT+j] = A[d][ch(g,j,s), n]
        apack = np.zeros((128, 64), f32)
        convw = np.zeros((128, 16), f32)
        ddiag = np.zeros((4, 128, 128), f32)
        for d in range(2):
            Ad = A[d][sl]
            cwd = cw[d][sl]
            for g in range(NG):
                for j in range(NT):
                    for s in range(8):
                        ch = g * 128 + 8 * j + s
                        apack[16 * s:16 * s + 16,
                              (d * NG + g) * NT + j] = Ad[ch]
                for k in range(KC):
                    convw[:, (d * NG + g) * KC + k] = \
                        cwd[g * 128:(g + 1) * 128, k]
            D_d = np.asarray(inputs[f'D_{"fwd" if d == 0 else "rev"}'],
                             f32)[sl]
            for g in range(NG):
                np.fill_diagonal(ddiag[d * NG + g],
                                 D_d[g * 128:(g + 1) * 128])
        bdt = np.zeros((128, 4), f32)
        convb = np.zeros((128, 4), f32)
        for d, nm in ((0, 'fwd'), (1, 'rev')):
            bd = np.asarray(inputs[f'bdt_{nm}'], f32)[sl]
            cb = np.asarray(inputs[f'conv_b_{nm}'], f32)[sl]
            for g in range(NG):
                bdt[:, 2 * d + g] = bd[g * 128:(g + 1) * 128]
                convb[:, 2 * d + g] = cb[g * 128:(g + 1) * 128]

        wxf = np.asarray(inputs['Wx_fwd'], f32)[:, sl].T   # [DLOC, 96]
        wxr = np.asarray(inputs['Wx_rev'], f32)[:, sl].T
        wxm = np.zeros((128, 384), f32)
        for g in range(NG):
            wxm[:, g * 192:g * 192 + 96] = wxf[g * 128:(g + 1) * 128]
            wxm[:, g * 192 + 96:g * 192 + 192] = wxr[g * 128:(g + 1) * 128]
        wxm = wxm.astype(BF16NP)
        woT = W_out[:, sl].T                               # [DLOC, DM]
        wom = np.concatenate([woT[g * 128:(g + 1) * 128]
                              for g in range(NG)], axis=1).astype(BF16NP)
        m = {
            'h_T': h_T,
            'w_in_T': np.ascontiguousarray(np.concatenate(
                [W_in[sl].T, W_in[DI + core * DLOC:DI + (core + 1) * DLOC].T],
                axis=1)).astype(BF16NP),
            'wx_T': wxm,
            'wdt_T': np.ascontiguousarray(np.concatenate(
                [np.asarray(inputs['Wdt_fwd'], f32)[sl].T,
                 np.asarray(inputs['Wdt_rev'], f32)[sl].T],
                axis=1)).astype(BF16NP),
            'w_out_T': wom,
            'sel01': sel01,
            'gsel': gsel,
            'selBC': selBC,
            'ddiag': np.concatenate(list(ddiag), axis=1).astype(BF16NP),
            'conv_w': convw,
            'apack': apack,
            'bdt': bdt,
            'conv_b': convb,
        }
        maps.append(m)
    return maps


def get_nc():
    if 'nc' not in _CACHE:
        _CACHE['nc'] = _build()
    return _CACHE['nc']


def run(inputs, **kw):
    nc = get_nc()
    maps = prepare_in_maps(inputs)
    res = run_bass_kernel_spmd(nc, maps, list(range(NCORES)), **kw)
    out_T = np.concatenate([np.asarray(res.results[c]['out_mT'],
                                       dtype=np.float32)
                            for c in range(NCORES)], axis=0)   # [DM, L]
    out = np.ascontiguousarray(out_T.T)[None]            # [1, L, DM]
    return out.astype(np.float32), res


def kernel(**inputs):
    out, _ = run(inputs)
    return out
